# revision 7
# baseline (speedup 1.0000x reference)
"""MedianBlur 3x3 raw-Bass v6: pair-shared vertical + fused E/O bands.

Changes vs v5 (133us baseline):
- Vertical sort3 shares the adjacent-row min/max pair between the two
  windows that straddle it (windows 2t and 2t+1 both use pair
  (2t, 2t+1)): 5 ops/row of elements instead of 6, emitted as 2 pair
  ops on K/2 rows + 4 dual-parity window ops on K rows (a [3*RW, 2]
  AP dim for the singles, stride-0 broadcast for the pairs).
- Horizontal stage fuses E/O band op pairs into single 4D-AP ops
  (band dim stride 258/259, or stride 0 for shared operands), using
  min/max commutativity: 12 ops instead of 20 per pass.
- Pass order K2(img0a) / K4(img5) / K16(imgs1-4) / K2(img0b): small
  first pass starts the DVE early, the big K16 load gets a long
  window, small last pass shrinks the store tail.
- No dma_reset preamble (end-of-block drains leave queues clean);
  start reset is sem_clear + barrier only.
"""

import os

import numpy as np

import concourse.bacc as bacc
import concourse.bass as bass
import concourse.mybir as mybir
from concourse.bass_utils import run_bass_kernel_spmd

BF16 = mybir.dt.bfloat16
MIN = mybir.AluOpType.min
MAX = mybir.AluOpType.max

N_CORES = 8
B, C, H, W = 16, 3, 512, 512
IMGS = (B // N_CORES) * C  # 6
HP = H + 2
PW = 258          # plane width (257 valid + 1 pad)
RW = 2 * PW       # row stride [E|O] = 516
HALF = 256        # valid outputs per plane row

# (K rows/partition, img, rowbase); all passes use 128 partitions
PASSES = [(2, 0, 0), (4, 5, 0), (16, 1, 0), (2, 0, 256)]
HH_OF = [0, 1, 2, 0]
NP = len(PASSES)
LAST = NP - 1

_cache = {}


def _ap(handle, off, dims):
    """Free-dim AP over all 128 partitions of an sbuf tensor."""
    free = handle.shape[1]
    return bass.AP(handle, off, [[free, 128]] + dims)


def _median_pass(V, Xf, Pn, Px, Lb, Hh, Mb, Tb, K):
    """Vertical: 6 ops, 5K*RW elems. Horizontal: 12 ops, 20*K*HALF elems."""
    Kh = K // 2

    # --- vertical stage: shared-pair sort3 ---
    # tile row i = image row r0-1+i; out row i lives at buffer row i.
    # pair t = image rows (r0+2t, r0+2t+1) = tile rows (2t+1, 2t+2);
    # it serves window 2t (single = tile row 2t) and window 2t+1
    # (single = tile row 2t+3).
    pair_a = _ap(Xf, RW, [[2 * RW, Kh], [1, RW]])
    pair_b = _ap(Xf, 2 * RW, [[2 * RW, Kh], [1, RW]])
    pn3 = _ap(Pn, 0, [[RW, Kh], [1, RW]])
    px3 = _ap(Px, 0, [[RW, Kh], [1, RW]])
    V.tensor_tensor(pn3, pair_a, pair_b, op=MIN)
    V.tensor_tensor(px3, pair_a, pair_b, op=MAX)

    S = _ap(Xf, 0, [[2 * RW, Kh], [3 * RW, 2], [1, RW]])
    pnb = _ap(Pn, 0, [[RW, Kh], [0, 2], [1, RW]])
    pxb = _ap(Px, 0, [[RW, Kh], [0, 2], [1, RW]])

    def winview(T):
        return _ap(T, 0, [[2 * RW, Kh], [RW, 2], [1, RW]])

    Lw, Hw, Mw = winview(Lb), winview(Hh), winview(Mb)
    V.tensor_tensor(Lw, S, pnb, op=MIN)   # lo  = min(s, Pn)
    V.tensor_tensor(Hw, S, pxb, op=MAX)   # hi  = max(s, Px)
    V.tensor_tensor(Mw, S, pxb, op=MIN)   # t   = min(s, Px)
    V.tensor_tensor(Mw, Mw, pnb, op=MAX)  # mid = max(t, Pn)

    # --- horizontal stage, E/O band pairs fused ---
    def b1(T, off):          # single band, K rows x 256
        return _ap(T, off, [[RW, K], [1, HALF]])

    def b2(T, off0, off1):   # band pair (slot0, slot1), K x 2 x 256
        return _ap(T, off0, [[RW, K], [off1 - off0, 2], [1, HALF]])

    def bs(T, off):          # shared operand broadcast over band dim
        return _ap(T, off, [[RW, K], [0, 2], [1, HALF]])

    E, E1, O, O1 = 0, 1, PW, PW + 1
    # A = max3_h(L) -> X bands: even win (LE,LO,LE1), odd (LO,LE1,LO1)
    V.tensor_tensor(b1(Tb, E), b1(Lb, O), b1(Lb, E1), op=MAX)          # mA
    V.tensor_tensor(b2(Xf, E, O), b2(Lb, E, O1), bs(Tb, E), op=MAX)    # A
    # C = min3_h(H) -> L bands
    V.tensor_tensor(b1(Tb, O), b1(Hh, O), b1(Hh, E1), op=MIN)          # mC
    V.tensor_tensor(b2(Lb, E, O), b2(Hh, E, O1), bs(Tb, O), op=MIN)    # C
    # B = med3_h(M) -> H bands (shared middle pair OP = (MO, ME1))
    V.tensor_tensor(b1(Tb, E), b1(Mb, O), b1(Mb, E1), op=MIN)          # OPn
    V.tensor_tensor(b1(Tb, O), b1(Mb, O), b1(Mb, E1), op=MAX)          # OPx
    V.tensor_tensor(b2(Hh, E, O), b2(Mb, E, O1), bs(Tb, O), op=MIN)    # t
    V.tensor_tensor(b2(Hh, E, O), b2(Hh, E, O), bs(Tb, E), op=MAX)     # B
    # final med3(A=X, B=H, C=L) -> H bands
    V.tensor_tensor(b2(Mb, E, O), b2(Xf, E, O), b2(Hh, E, O), op=MIN)  # U
    V.tensor_tensor(b2(Xf, E, O), b2(Xf, E, O), b2(Hh, E, O), op=MAX)  # V
    V.tensor_tensor(b2(Xf, E, O), b2(Xf, E, O), b2(Lb, E, O), op=MIN)  # W
    return V.tensor_tensor(b2(Hh, E, O), b2(Mb, E, O), b2(Xf, E, O), op=MAX)


def _build():
    nc = bacc.Bacc(
        "TRN2", target_bir_lowering=False, debug=False, num_devices=N_CORES
    )
    xp = nc.declare_dram_parameter("xp", [IMGS, HP, RW], BF16, isOutput=False)
    y = nc.declare_dram_parameter("y", [IMGS, H, W], BF16, isOutput=True)

    Xs = [
        nc.alloc_sbuf_tensor(f"X{i}", [128, (K + 2) * RW], BF16)
        for i, (K, _, _) in enumerate(PASSES)
    ]
    Pn = nc.alloc_sbuf_tensor("Pn", [128, 8 * RW], BF16)
    Px = nc.alloc_sbuf_tensor("Px", [128, 8 * RW], BF16)
    Lb = nc.alloc_sbuf_tensor("Lb", [128, 16 * RW], BF16)
    Mb = nc.alloc_sbuf_tensor("Mb", [128, 16 * RW], BF16)
    Tb = nc.alloc_sbuf_tensor("Tb", [128, 16 * RW], BF16)
    hh_k = [
        max(PASSES[p][0] for p in range(NP) if HH_OF[p] == b) for b in range(3)
    ]
    Hhs = [
        nc.alloc_sbuf_tensor(f"Hh{b}", [128, hh_k[b] * RW], BF16) for b in range(3)
    ]

    def load_ap(ps, p0, npart):
        K, img, rowbase = PASSES[ps]
        pimg = H // K
        img = img + p0 // pimg
        row0 = rowbase + (p0 % pimg) * K
        return bass.AP(
            xp,
            img * HP * RW + row0 * RW,
            [[K * RW, npart], [1, (K + 2) * RW]],
        )

    def store_aps(ps, p0, npart):
        K, img, rowbase = PASSES[ps]
        pimg = H // K
        img = img + p0 // pimg
        row0 = rowbase + (p0 % pimg) * K
        dst = bass.AP(y, img * H * W + row0 * W, [[K * W, npart], [1, K * W]])
        src = Hhs[HH_OF[ps]][p0 : p0 + npart, :].rearrange(
            "p (r b c) -> p r b c", b=2, c=PW
        )[:, 0:K, :, 0:HALF]
        return dst, src

    load_sems = [nc.alloc_semaphore(f"pload{i}") for i in range(NP)]
    dve_sem = nc.alloc_semaphore("pdve_sem")
    stA = nc.alloc_semaphore("pstA")  # pass-0 stores (gates Hh0 reuse)
    stB = nc.alloc_semaphore("pstB")  # pass 1..3 stores

    nums = sorted(h.num for h in load_sems + [dve_sem, stA, stB])
    lo, hi = nums[0], nums[-1]
    assert nums == list(range(lo, hi + 1)), nums
    nc.sync.sem_clear(range(lo, hi + 1))
    nc.all_engine_barrier()

    # (pass, p0, npart) per trigger engine; each chunk incs its sem by 16.
    # Chunks never span an image boundary (DRAM rows are HP=514 per image,
    # so a linear [K*RW, npart] walk breaks at img edges).  "wait" entries
    # stall the *trigger* so a big load can't starve an urgent one (HWDGE
    # queues round-robin packets across outstanding DMAs).
    LOADS = {
        "sync": [(0, 0, 43), ("wait", 0), (1, 0, 43), ("wait", 1),
                 (2, 64, 32), (3, 0, 64)],
        "scalar": [(0, 43, 43), ("wait", 0), (1, 43, 43), ("wait", 1),
                   (2, 96, 32), (3, 64, 64)],
        "gpsimd": [(0, 86, 42), ("wait", 0), (1, 86, 42), ("wait", 1),
                   (2, 0, 32), (2, 32, 32)],
    }
    LOAD_THRESH = [48, 48, 64, 32]
    STORES = {
        "sync": [(0, 0, 64), (1, 0, 64), (2, 0, 32), (2, 64, 32), (3, 0, 32)],
        "scalar": [(0, 64, 64), (1, 64, 64), (2, 32, 32), (2, 96, 32),
                   (3, 32, 32)],
        "gpsimd": [(3, 64, 64)],
    }
    N_STA = 2           # pass-0 store chunks
    N_STB = 9           # pass 1..3 store chunks

    def emit_loads(eng, name):
        for entry in LOADS[name]:
            if entry[0] == "wait":
                eng.wait_ge(load_sems[entry[1]], LOAD_THRESH[entry[1]])
                continue
            ps, p0, npart = entry
            eng.dma_start(
                out=Xs[ps][p0 : p0 + npart, :], in_=load_ap(ps, p0, npart)
            ).then_inc(load_sems[ps], 16)

    def emit_stores(eng, name):
        cur = 0
        for ps, p0, npart in STORES[name]:
            if ps + 1 > cur:
                cur = ps + 1
                eng.wait_ge(dve_sem, cur)
            dst, src = store_aps(ps, p0, npart)
            sem = stA if ps == 0 else stB
            eng.dma_start(out=dst, in_=src).then_inc(sem, 16)

    with nc.Block() as blk:

        @blk.sync
        def _(sync):
            emit_loads(sync, "sync")
            emit_stores(sync, "sync")
            sync.wait_ge(stA, N_STA * 16)
            sync.wait_ge(stB, N_STB * 16)

        @blk.scalar
        def _(scalar):
            emit_loads(scalar, "scalar")
            emit_stores(scalar, "scalar")

        @blk.gpsimd
        def _(gp):
            emit_loads(gp, "gpsimd")
            emit_stores(gp, "gpsimd")

        @blk.vector
        def _(V):
            for ps, (K, img, rowbase) in enumerate(PASSES):
                V.wait_ge(load_sems[ps], LOAD_THRESH[ps])
                if ps == LAST:
                    V.wait_ge(stA, N_STA * 16)  # Hh0 reuse
                _median_pass(
                    V, Xs[ps], Pn, Px, Lb, Hhs[HH_OF[ps]], Mb, Tb, K
                ).then_inc(dve_sem, 1)

    nc.finalize()
    return nc


LAST_EXEC_TIME_NS = None
LAST_TRACE = None


def _to_bf16_u16(a: np.ndarray) -> np.ndarray:
    u = a.view(np.uint32)
    r = ((u >> 16) & np.uint32(1)) + np.uint32(0x7FFF)
    return ((u + r) >> 16).astype(np.uint16)


def run(x: np.ndarray, trace: bool = False):
    global LAST_EXEC_TIME_NS, LAST_TRACE
    assert x.shape == (B, C, H, W), x.shape
    x = np.ascontiguousarray(x, dtype=np.float32)

    import ml_dtypes

    if "P" not in _cache:
        _cache["P"] = _build()
    nc = _cache["P"]

    xpad = np.pad(x, ((0, 0), (0, 0), (1, 1), (1, 1)))  # (B,C,514,514)
    planes = np.zeros((B, C, HP, 2, PW), dtype=np.float32)
    planes[..., 0, :257] = xpad[..., 0::2]
    planes[..., 1, :257] = xpad[..., 1::2]
    xb = _to_bf16_u16(np.ascontiguousarray(planes)).view(ml_dtypes.bfloat16)
    shards = xb.reshape(N_CORES, IMGS, HP, RW)
    in_maps = [{"xp": shards[c]} for c in range(N_CORES)]

    if not trace:
        os.environ["BASS_NEVER_TRACE"] = "1"
    else:
        os.environ.pop("BASS_NEVER_TRACE", None)
    res = run_bass_kernel_spmd(nc, in_maps, list(range(N_CORES)), trace=trace)
    LAST_EXEC_TIME_NS = res.exec_time_ns
    LAST_TRACE = res.instructions_and_trace
    yp = np.stack(
        [np.asarray(res.results[c]["y"]).astype(np.float32) for c in range(N_CORES)]
    ).reshape(B, C, H, 2, HALF)
    out = np.empty((B, C, H, W), dtype=np.float32)
    out[..., 0::2] = yp[..., 0, :]
    out[..., 1::2] = yp[..., 1, :]
    return out


def kernel(x: np.ndarray) -> np.ndarray:
    return run(x, trace=False)
